# revision 13
# baseline (speedup 1.0000x reference)
"""Trainium2 Bass kernel for nn_BitwiseMLP: 3x (Linear + training-mode BatchNorm).

Math: per layer, h = gamma * (y - mean_B(y)) * rsqrt(var_B(y) + eps) + beta with
y = x @ W.T + b. BatchNorm is invariant to per-feature constant shifts of y, so
every linear bias and the additive part of each BN affine cancels; only the
multiplicative scales a_l = gamma_l * rsqrt(var_l + eps) propagate (applied to
the next layer's input activations), plus one final affine a2*u2 + c2.

Device layout: transposed activations [features, batch_rows]; batch sharded 8
ways (2048 rows/core); weights replicated, strip-major pre-tiled on host.
Matmuls bf16 (fp32 PSUM), stats fp32, one small AllReduce per stats chunk.

Schedule (the perf-critical part):
 - Cross-core stats flow is split: part1 (bn_aggr -> S/Q -> AllReduce issue)
   is emitted inline at each chunk boundary; part2 (readback -> rstd -> scales)
   is emitted ~2 strips later so the DVE queue never head-of-line blocks on a
   collective in flight (that HOL block cost the old kernel ~8.5us per layer
   boundary).
 - The last stats chunk of L0/L1 (2 strips) completes only when the layer's PE
   work ends, so layers 1/2 interleave their first two m-strips: both run
   k=0..13 first, giving the previous layer's final AllReduce a ~30us runway
   before the deferred k-tiles 14-15 need the scaled activations.
 - Layer 2 output is bf16 (error budget allows it), chunked [3,2,2,1] with
   scale+writeback per chunk on two DMA rings; the last strip runs k-chained
   (n-outer) so its stats start before the final matmul retires, and the
   exposed tail is one strip's stats + AllReduce + 512KB writeback.
 - DMA ring assignment: sync = x halves + stats traffic + out half0;
   scalar(Act) = x halves only (keeps the Act queue free for PSUM copies);
   gpsimd = weight strips (prefetch depth 3) + collective triggers + out half1.
"""

import numpy as np
import ml_dtypes

# ---- problem constants (full size; hardcoded per harness contract) ----
N_CORES = 8
B_FULL = 16384
D_IN = 1024
D_H = 2048
D_OUT = 1024
BN_EPS = 1e-5

_PROG_CACHE = {}
LAST_RESULTS = None  # BassKernelResults of the most recent run (for test harness)


def dedup_ldweights(nc, mybir):
    """Remove InstLdweights that reload the exact weight tile already resident
    in the PE array (same memref/offset/pattern, nothing clobbering between).

    tile_legalize splits every non-f32 matmul into LDWEIGHTS+MATMUL pairs even
    when consecutive matmuls share weights; letting same-weight matmuls run
    back-to-back keeps the PE's shadow weight load off the critical path.

    Safety: a removed LDW's waits/updates/deps move onto its paired matmul,
    and we only remove when the resulting matmul has <=1 wait so compile's
    move_matmul_waits_to_ldweights cannot hoist a wait onto the shared LDW
    (which would risk deadlock).
    """
    PE = mybir.EngineType.PE
    n_removed = 0
    for b in nc.main_func.blocks:
        last_key = None
        insts = b.instructions
        remove = set()
        i, n = 0, len(insts)
        while i < n:
            inst = insts[i]
            tn = type(inst).__name__
            if getattr(inst, "engine", None) != PE:
                i += 1
                continue
            if tn == "InstLdweights":
                a = inst.ins[0]
                try:
                    key = (a.memref, a.offset, str(a.ap), str(a.dtype),
                           str(inst.perf_mode), str(inst.is_transpose),
                           str(inst.tile_position))
                except Exception:
                    key = None
                if key is not None and key == last_key:
                    j = i + 1
                    mm = None
                    while j < n:
                        nx = insts[j]
                        if getattr(nx, "engine", None) == PE:
                            tnx = type(nx).__name__
                            if tnx == "InstMatmult":
                                mm = nx
                                break
                            if tnx == "InstLdweights":
                                break
                        j += 1
                    si = inst.sync_info
                    waits = list(si.on_wait) if si else []
                    upds = list(si.on_update) if si else []
                    mmsi = mm.sync_info if mm is not None else None
                    mm_waits = list(mmsi.on_wait) if mmsi else []
                    if mm is not None and len(mm_waits) + len(waits) <= 1:
                        if waits or upds:
                            if mmsi is None:
                                mm.sync_info = mybir.SyncInfo(
                                    on_wait=waits, on_update=upds)
                            else:
                                mm.sync_info = mybir.SyncInfo(
                                    on_wait=mm_waits + waits,
                                    on_update=list(mmsi.on_update) + upds)
                        mm.merge_dependencies_from(inst)
                        remove.add(i)
                        n_removed += 1
                        i += 1
                        continue
                last_key = key
            elif tn == "InstMatmult":
                if getattr(inst, "is_transpose", False):
                    last_key = None
            i += 1
        if remove:
            removed_names = {insts[k].name for k in remove}
            b.instructions = [x for k, x in enumerate(insts) if k not in remove]
            for bb in nc.main_func.blocks:
                for x in bb.instructions:
                    for nm in removed_names:
                        try:
                            x.try_remove_dependency(nm)
                        except Exception:
                            pass
    return n_removed


def build_program(R, B_total):
    """Build the per-core Bass program. R = batch rows per core."""
    import contextlib

    import concourse.bacc as bacc
    import concourse.mybir as mybir
    import concourse.tile as tile

    f32 = mybir.dt.float32
    bf16 = mybir.dt.bfloat16
    Alu = mybir.AluOpType
    Act = mybir.ActivationFunctionType

    assert R % 512 == 0
    NT = R // 512
    KT = [D_IN // 128, D_H // 128, D_H // 128]
    MT = [D_H // 128, D_H // 128, D_OUT // 128]
    assert MT[2] == 8 and KT[0] == 8 and NT == 4, "schedule tuned for full size"
    inv_B = 1.0 / float(B_total)
    GROUP = [list(range(N_CORES))]
    HEAD = 2                      # strips interleaved at L1/L2 start
    SPLIT = [KT[l] - 2 for l in range(3)]  # deferred k-tail for head strips
    CHB = [[0, 8, 14, MT[0]],
           [0, 8, 14, MT[1]],
           [0, 3, 5, 7, 8]]   # L2 output-strip chunks

    nc = bacc.Bacc(None, num_devices=N_CORES)

    xt_d = nc.dram_tensor("xt", [D_IN, R], bf16, kind="ExternalInput")
    # weights pre-tiled strip-major on host: [m_strip, partition(k%128), k//128*128+f]
    w0_d = nc.dram_tensor("w0t", [MT[0], 128, KT[0] * 128], bf16, kind="ExternalInput")
    w1_d = nc.dram_tensor("w1t", [MT[1], 128, KT[1] * 128], bf16, kind="ExternalInput")
    w2_d = nc.dram_tensor("w2t", [MT[2], 128, KT[2] * 128], bf16, kind="ExternalInput")
    g0_d = nc.dram_tensor("g0", [D_H], f32, kind="ExternalInput")
    g1_d = nc.dram_tensor("g1", [D_H], f32, kind="ExternalInput")
    g2_d = nc.dram_tensor("g2", [D_OUT], f32, kind="ExternalInput")
    b2_d = nc.dram_tensor("beta2", [D_OUT], f32, kind="ExternalInput")
    out_d = nc.dram_tensor("out", [D_OUT, R], bf16, kind="ExternalOutput")

    cc_in = [[nc.dram_tensor(f"cc_in{l}_{q}", [128, 2 * (b - a)], f32)
              for q, (a, b) in enumerate(zip(CHB[l], CHB[l][1:]))]
             for l in range(3)]
    cc_out = [[nc.dram_tensor(f"cc_out{l}_{q}", [128, 2 * (b - a)], f32,
                              addr_space="Shared")
               for q, (a, b) in enumerate(zip(CHB[l], CHB[l][1:]))]
              for l in range(3)]

    with tile.TileContext(nc) as tc:
        with contextlib.ExitStack() as ctx:
            act = ctx.enter_context(tc.tile_pool(name="act", bufs=32))
            w0p = ctx.enter_context(tc.tile_pool(name="w0p", bufs=8))
            wp = ctx.enter_context(tc.tile_pool(name="wp", bufs=4))
            pspool = ctx.enter_context(tc.tile_pool(name="ps", bufs=8, space="PSUM"))
            small = ctx.enter_context(tc.tile_pool(name="small", bufs=1))

            # ---------- initial loads ----------
            xt_r = xt_d[:].rearrange("(j p) r -> p j r", p=128)
            H = R // 2
            w0s = []
            for m in range(2):
                t = w0p.tile([128, KT[0] * 128], bf16, tag="w0", name=f"w0_{m}")
                nc.gpsimd.dma_start(out=t, in_=w0_d[m])
                w0s.append(t)
            GP_X = {2, 5}  # these x k-tiles ride the gpsimd ring whole
            xts = []
            for j in range(KT[0]):
                t = act.tile([128, R], bf16, tag="act", name=f"xt{j}")
                if j in GP_X:
                    nc.gpsimd.dma_start(out=t, in_=xt_r[:, j, :])
                else:
                    nc.sync.dma_start(out=t[:, 0:H], in_=xt_r[:, j, 0:H])
                    nc.scalar.dma_start(out=t[:, H:R], in_=xt_r[:, j, H:R])
                xts.append(t)
            for m in range(2, MT[0]):
                t = w0p.tile([128, KT[0] * 128], bf16, tag="w0", name=f"w0_{m}")
                nc.gpsimd.dma_start(out=t, in_=w0_d[m])
                w0s.append(t)

            g_t = []
            for l, gd in enumerate((g0_d, g1_d, g2_d)):
                t = small.tile([128, MT[l]], f32, tag=f"g{l}", name=f"g{l}")
                nc.gpsimd.dma_start(out=t, in_=gd[:].rearrange("(m p) -> p m", p=128))
                g_t.append(t)
            b2_t = small.tile([128, MT[2]], f32, tag="b2", name="b2")
            nc.gpsimd.dma_start(out=b2_t, in_=b2_d[:].rearrange("(m p) -> p m", p=128))

            eps_t = small.tile([128, 1], f32, tag="eps", name="eps")
            nc.vector.memset(eps_t, BN_EPS)
            # prime the Act engine's sqrt table outside the critical path
            dummy = small.tile([128, 1], f32, tag="dummy", name="dummy")
            nc.vector.memset(dummy, 1.0)
            nc.scalar.activation(out=dummy, in_=dummy, func=Act.Sqrt,
                                 bias=eps_t[:, 0:1])

            # w1 strip prefetch (behind w0 on the gpsimd ring)
            w1s = {}

            def w1_tile(m):
                if m not in w1s:
                    t = wp.tile([128, KT[1] * 128], bf16, tag="w", name=f"w1_{m}")
                    nc.gpsimd.dma_start(out=t, in_=w1_d[m])
                    w1s[m] = t
                return w1s[m]

            w2s = {}

            def w2_tile(m):
                if m not in w2s:
                    t = wp.tile([128, KT[2] * 128], bf16, tag="w", name=f"w2_{m}")
                    nc.gpsimd.dma_start(out=t, in_=w2_d[m])
                    w2s[m] = t
                return w2s[m]

            for m in range(3):
                w1_tile(m)

            BN = [small.tile([128, MT[l] * NT * 6], f32, tag=f"BN{l}", name=f"BN{l}")
                  for l in range(3)]

            # ---------- stats helpers ----------
            def part1(l, q):
                """pre-AR: aggregate chunk stats, form S/Q sums, AllReduce."""
                m0, m1 = CHB[l][q], CHB[l][q + 1]
                mh = m1 - m0
                mv = small.tile([128, mh, 2], f32, tag=f"mv{l}{q}", name=f"mv{l}{q}")
                for m in range(m0, m1):
                    nc.vector.bn_aggr(out=mv[:, m - m0, :],
                                      in_=BN[l][:, m * NT * 6:(m + 1) * NT * 6])
                # S = mean*R ; Q = (var + mean^2)*R  (exact cross-core sums)
                sf = small.tile([128, 2, mh], f32, tag=f"sf{l}{q}", name=f"sf{l}{q}")
                nc.vector.tensor_scalar_mul(sf[:, 0, :], mv[:, :, 0], float(R))
                nc.vector.tensor_mul(sf[:, 1, :], mv[:, :, 0], mv[:, :, 0])
                nc.vector.tensor_add(sf[:, 1, :], sf[:, 1, :], mv[:, :, 1])
                nc.vector.tensor_scalar_mul(sf[:, 1, :], sf[:, 1, :], float(R))
                nc.sync.dma_start(out=cc_in[l][q][:], in_=sf)
                nc.gpsimd.collective_compute(
                    "AllReduce", Alu.add, replica_groups=GROUP,
                    ins=[cc_in[l][q][:]], outs=[cc_out[l][q][:]])

            def part2(l, q, want_c):
                """post-AR: readback -> mean/var -> a = gamma*rstd [, c]."""
                m0, m1 = CHB[l][q], CHB[l][q + 1]
                mh = m1 - m0
                sg = small.tile([128, 2, mh], f32, tag=f"sg{l}{q}", name=f"sg{l}{q}")
                nc.sync.dma_start(
                    out=sg, in_=cc_out[l][q][:].rearrange("p (s m) -> p s m", s=2))
                mean = small.tile([128, mh], f32, tag=f"mean{l}{q}", name=f"mean{l}{q}")
                var = small.tile([128, mh], f32, tag=f"var{l}{q}", name=f"var{l}{q}")
                tmp = small.tile([128, mh], f32, tag=f"tmp{l}{q}", name=f"tmp{l}{q}")
                nc.vector.tensor_scalar_mul(mean, sg[:, 0, :], inv_B)
                nc.vector.tensor_scalar_mul(var, sg[:, 1, :], inv_B)
                nc.vector.tensor_mul(tmp, mean, mean)
                nc.vector.tensor_sub(var, var, tmp)
                nc.scalar.activation(out=var, in_=var, func=Act.Sqrt,
                                     bias=eps_t[:, 0:1])
                nc.vector.reciprocal(out=var, in_=var)
                a = small.tile([128, mh], f32, tag=f"a{l}{q}", name=f"a{l}{q}")
                nc.vector.tensor_mul(a, var, g_t[l][:, m0:m1])
                if not want_c:
                    return a, None
                c = small.tile([128, mh], f32, tag=f"c{l}{q}", name=f"c{l}{q}")
                nc.vector.tensor_mul(tmp, a, mean)
                nc.vector.tensor_sub(c, b2_t[:, m0:m1], tmp)
                return a, c

            def post(l, q, strips):
                """part2 + in-place u *= a for the chunk; 3/4 DVE, 1/4 Act."""
                a, _ = part2(l, q, False)
                m0, m1 = CHB[l][q], CHB[l][q + 1]
                for m in range(m0, m1):
                    s = strips[m][:]
                    ac = a[:, m - m0:m - m0 + 1]
                    if m % 4 == 3:
                        nc.scalar.activation(out=s, in_=s, func=Act.Copy, scale=ac)
                    else:
                        nc.vector.tensor_scalar_mul(s, s, ac)

            # ---------- matmul helpers ----------
            def w_ap_of(t):
                return lambda j: t[:, j * 128:(j + 1) * 128]

            def strips_rhs(strips):
                return lambda j, n: strips[j][:, n * 512:(n + 1) * 512]

            def mm_group(l, pss, w_ap, rhs, j):
                for n in range(NT):
                    nc.tensor.matmul(pss[n], w_ap(j), rhs(j, n),
                                     start=(j == 0), stop=(j == KT[l] - 1),
                                     skip_group_check=True)

            def consumers(l, m, pss, dest_at):
                for n in range(NT):
                    idx = m * NT + n
                    nc.scalar.activation(out=dest_at(m, n), in_=pss[n],
                                         func=Act.Copy)
                    nc.vector.bn_stats(out=BN[l][:, idx * 6:idx * 6 + 6],
                                       in_=pss[n])

            def ps_tiles(l, m):
                return [pspool.tile([128, 512], f32, tag="ps",
                                    name=f"ps{l}_{m}_{n}") for n in range(NT)]

            # ================= layer 0 (plain m loop) =================
            u0 = [act.tile([128, R], bf16, tag="act", name=f"u0_{m}")
                  for m in range(MT[0])]

            def u0_at(m, n):
                return u0[m][:, n * 512:(n + 1) * 512]

            rhs0 = strips_rhs(xts)
            for m in range(MT[0]):
                pss = ps_tiles(0, m)
                wap = w_ap_of(w0s[m])
                for j in range(KT[0]):
                    mm_group(0, pss, wap, rhs0, j)
                consumers(0, m, pss, u0_at)
                if m == CHB[0][1] - 1:
                    part1(0, 0)
                if m == CHB[0][2] - 1:
                    part1(0, 1)
                if m == CHB[0][2]:
                    post(0, 0, u0)

            # ================= layer 1 (interleaved head) =================
            part1(0, 2)  # L0 final chunk: start its AllReduce ASAP
            post(0, 1, u0)
            u1 = [act.tile([128, R], bf16, tag="act", name=f"u1_{m}")
                  for m in range(MT[1])]

            def u1_at(m, n):
                return u1[m][:, n * 512:(n + 1) * 512]

            rhs1 = strips_rhs(u0)
            head_ps = []
            for m in range(HEAD):
                w1_tile(m + 3)
                t = w1_tile(m)
                pss = ps_tiles(1, m)
                head_ps.append((pss, t))
                for j in range(SPLIT[1]):
                    mm_group(1, pss, w_ap_of(t), rhs1, j)
            post(0, 2, u0)  # deferred scales for u0[12..15]
            for m in range(HEAD):
                pss, t = head_ps[m]
                for j in range(SPLIT[1], KT[1]):
                    mm_group(1, pss, w_ap_of(t), rhs1, j)
                consumers(1, m, pss, u1_at)
            for m in range(HEAD, MT[1]):
                if m + 3 < MT[1]:
                    w1_tile(m + 3)
                else:
                    w2_tile(m + 3 - MT[1])  # prefetch w2 strips 0..2
                pss = ps_tiles(1, m)
                wap = w_ap_of(w1_tile(m))
                for j in range(KT[1]):
                    mm_group(1, pss, wap, rhs1, j)
                consumers(1, m, pss, u1_at)
                if m == CHB[1][1] - 1:
                    part1(1, 0)
                if m == CHB[1][2] - 1:
                    part1(1, 1)
                if m == CHB[1][2]:
                    post(1, 0, u1)

            # ================= layer 2 (interleaved head, bf16 out) =========
            part1(1, 2)
            post(1, 1, u1)
            u2 = [act.tile([128, R], bf16, tag="act", name=f"u2_{m}")
                  for m in range(MT[2])]

            def u2_at(m, n):
                return u2[m][:, n * 512:(n + 1) * 512]

            def post2(q, last=False):
                """part2 + final affine (bf16, DVE) + writeback on two rings."""
                a, c = part2(2, q, True)
                m0, m1 = CHB[2][q], CHB[2][q + 1]
                for m in range(m0, m1):
                    am = a[:, m - m0:m - m0 + 1]
                    cm = c[:, m - m0:m - m0 + 1]
                    s = u2[m][:]
                    nc.vector.tensor_scalar(s, s, am, cm, Alu.mult, Alu.add)
                    nc.sync.dma_start(
                        out=out_d[m * 128:(m + 1) * 128, 0:H], in_=u2[m][:, 0:H])
                    nc.gpsimd.dma_start(
                        out=out_d[m * 128:(m + 1) * 128, H:R], in_=u2[m][:, H:R])

            rhs2 = strips_rhs(u1)
            head_ps = []
            for m in range(HEAD):
                w2_tile(m + 3)
                t = w2_tile(m)
                pss = ps_tiles(2, m)
                head_ps.append((pss, t))
                for j in range(SPLIT[2]):
                    mm_group(2, pss, w_ap_of(t), rhs2, j)
            post(1, 2, u1)  # deferred scales for u1[12..15]
            for m in range(HEAD):
                pss, t = head_ps[m]
                for j in range(SPLIT[2], KT[2]):
                    mm_group(2, pss, w_ap_of(t), rhs2, j)
                consumers(2, m, pss, u2_at)
            # boundary m -> (part1 chunk, post2 chunk) per CHB[2]=[0,3,5,7,8]
            bound = {2: (0, None), 4: (1, 0), 6: (2, 1), 7: (3, 2)}
            for m in range(HEAD, MT[2]):
                if m + 3 < MT[2]:
                    w2_tile(m + 3)
                pss = ps_tiles(2, m)
                wap = w_ap_of(w2_tile(m))
                if m == MT[2] - 1:
                    # last strip: k-chain per psum tile so stats/copies of
                    # early tiles start before the layer's final matmul
                    for n in range(NT):
                        for j in range(KT[2]):
                            nc.tensor.matmul(
                                pss[n], wap(j), rhs2(j, n),
                                start=(j == 0), stop=(j == KT[2] - 1),
                                skip_group_check=True)
                        idx = m * NT + n
                        nc.scalar.activation(out=u2_at(m, n), in_=pss[n],
                                             func=Act.Copy)
                        nc.vector.bn_stats(
                            out=BN[2][:, idx * 6:idx * 6 + 6], in_=pss[n])
                else:
                    for j in range(KT[2]):
                        mm_group(2, pss, wap, rhs2, j)
                    consumers(2, m, pss, u2_at)
                if m in bound:
                    p1q, p2q = bound[m]
                    part1(2, p1q)
                    if p2q is not None:
                        post2(p2q)
            post2(len(CHB[2]) - 2, last=True)

    dedup_ldweights(nc, mybir)
    nc.compile()
    return nc


def _get_program(R, B_total):
    key = (R, B_total)
    if key not in _PROG_CACHE:
        _PROG_CACHE[key] = build_program(R, B_total)
    return _PROG_CACHE[key]


def prep_inputs(x, W0, W1, W2, gamma0, gamma1, gamma2, beta2, n_cores=N_CORES):
    """Host-side: transpose, cast to bf16, pre-tile weights, shard batch."""
    bf = ml_dtypes.bfloat16

    def strip_tiles(W):
        # W [F, K] -> [F//128 strips, 128 partitions(k%128), (K//128)*128] bf16
        # element [m, p, j*128+f] = W[m*128+f, j*128+p]
        F, Kd = W.shape
        wt = W.T.reshape(Kd // 128, 128, F // 128, 128)  # [j, p, m, f]
        return np.ascontiguousarray(wt.transpose(2, 1, 0, 3)).reshape(
            F // 128, 128, Kd // 128 * 128
        ).astype(bf)

    xT = np.ascontiguousarray(x.T)  # [D_IN, B]
    R = x.shape[0] // n_cores
    w0t = strip_tiles(np.asarray(W0, dtype=np.float32))
    w1t = strip_tiles(np.asarray(W1, dtype=np.float32))
    w2t = strip_tiles(np.asarray(W2, dtype=np.float32))
    g0 = np.ascontiguousarray(gamma0, dtype=np.float32)
    g1 = np.ascontiguousarray(gamma1, dtype=np.float32)
    g2 = np.ascontiguousarray(gamma2, dtype=np.float32)
    b2 = np.ascontiguousarray(beta2, dtype=np.float32)
    in_maps = []
    for c in range(n_cores):
        in_maps.append(
            {
                "xt": np.ascontiguousarray(xT[:, c * R:(c + 1) * R]).astype(bf),
                "w0t": w0t,
                "w1t": w1t,
                "w2t": w2t,
                "g0": g0,
                "g1": g1,
                "g2": g2,
                "beta2": b2,
            }
        )
    return in_maps, R


def kernel(
    x,
    W0,
    b0,
    gamma0,
    beta0,
    W1,
    b1,
    gamma1,
    beta1,
    W2,
    b2,
    gamma2,
    beta2,
):
    """Full-input entry point: shard across 8 NeuronCores, run, gather.

    b0/b1/b2/beta0/beta1 cancel exactly under training-mode BatchNorm
    (shift invariance), so they are not shipped to the device.
    """
    global LAST_RESULTS
    from concourse.bass_utils import run_bass_kernel_spmd

    x = np.asarray(x, dtype=np.float32)
    B = x.shape[0]
    in_maps, R = prep_inputs(
        x, np.asarray(W0), np.asarray(W1), np.asarray(W2),
        np.asarray(gamma0), np.asarray(gamma1), np.asarray(gamma2),
        np.asarray(beta2),
    )
    nc = _get_program(R, B)
    res = run_bass_kernel_spmd(nc, in_maps, core_ids=list(range(N_CORES)))
    LAST_RESULTS = res
    out = np.empty((B, D_OUT), dtype=np.float32)
    for c in range(N_CORES):
        out[c * R:(c + 1) * R, :] = res.results[c]["out"].T.astype(np.float32)
    return out


# revision 14
# speedup vs baseline: 1.0097x; 1.0097x over previous
"""Trainium2 Bass kernel for nn_BitwiseMLP: 3x (Linear + training-mode BatchNorm).

Math: per layer, h = gamma * (y - mean_B(y)) * rsqrt(var_B(y) + eps) + beta with
y = x @ W.T + b. BatchNorm is invariant to per-feature constant shifts of y, so
every linear bias and the additive part of each BN affine cancels; only the
multiplicative scales a_l = gamma_l * rsqrt(var_l + eps) propagate (applied to
the next layer's input activations), plus one final affine a2*u2 + c2.

Device layout: transposed activations [features, batch_rows]; batch sharded 8
ways (2048 rows/core); weights replicated, strip-major pre-tiled on host.
Matmuls bf16 (fp32 PSUM), stats fp32, one small AllReduce per stats chunk.

Schedule (the perf-critical part):
 - Cross-core stats flow is split: part1 (bn_aggr -> S/Q -> AllReduce issue)
   is emitted inline at each chunk boundary; part2 (readback -> rstd -> scales)
   is emitted ~2 strips later so the DVE queue never head-of-line blocks on a
   collective in flight (that HOL block cost the old kernel ~8.5us per layer
   boundary).
 - The last stats chunk of L0/L1 (2 strips) completes only when the layer's PE
   work ends, so layers 1/2 interleave their first two m-strips: both run
   k=0..13 first, giving the previous layer's final AllReduce a ~30us runway
   before the deferred k-tiles 14-15 need the scaled activations.
 - Layer 2 output is bf16 (error budget allows it), chunked [3,2,2,1] with
   scale+writeback per chunk on two DMA rings; the last strip runs k-chained
   (n-outer) so its stats start before the final matmul retires, and the
   exposed tail is one strip's stats + AllReduce + 512KB writeback.
 - DMA ring assignment: sync = x halves + stats traffic + out half0;
   scalar(Act) = x halves only (keeps the Act queue free for PSUM copies);
   gpsimd = weight strips (prefetch depth 3) + collective triggers + out half1.
"""

import numpy as np
import ml_dtypes

# ---- problem constants (full size; hardcoded per harness contract) ----
N_CORES = 8
B_FULL = 16384
D_IN = 1024
D_H = 2048
D_OUT = 1024
BN_EPS = 1e-5

_PROG_CACHE = {}
LAST_RESULTS = None  # BassKernelResults of the most recent run (for test harness)


def dedup_ldweights(nc, mybir):
    """Remove InstLdweights that reload the exact weight tile already resident
    in the PE array (same memref/offset/pattern, nothing clobbering between).

    tile_legalize splits every non-f32 matmul into LDWEIGHTS+MATMUL pairs even
    when consecutive matmuls share weights; letting same-weight matmuls run
    back-to-back keeps the PE's shadow weight load off the critical path.

    Safety: a removed LDW's waits/updates/deps move onto its paired matmul,
    and we only remove when the resulting matmul has <=1 wait so compile's
    move_matmul_waits_to_ldweights cannot hoist a wait onto the shared LDW
    (which would risk deadlock).
    """
    PE = mybir.EngineType.PE
    n_removed = 0
    for b in nc.main_func.blocks:
        last_key = None
        insts = b.instructions
        remove = set()
        i, n = 0, len(insts)
        while i < n:
            inst = insts[i]
            tn = type(inst).__name__
            if getattr(inst, "engine", None) != PE:
                i += 1
                continue
            if tn == "InstLdweights":
                a = inst.ins[0]
                try:
                    key = (a.memref, a.offset, str(a.ap), str(a.dtype),
                           str(inst.perf_mode), str(inst.is_transpose),
                           str(inst.tile_position))
                except Exception:
                    key = None
                if key is not None and key == last_key:
                    j = i + 1
                    mm = None
                    while j < n:
                        nx = insts[j]
                        if getattr(nx, "engine", None) == PE:
                            tnx = type(nx).__name__
                            if tnx == "InstMatmult":
                                mm = nx
                                break
                            if tnx == "InstLdweights":
                                break
                        j += 1
                    si = inst.sync_info
                    waits = list(si.on_wait) if si else []
                    upds = list(si.on_update) if si else []
                    mmsi = mm.sync_info if mm is not None else None
                    mm_waits = list(mmsi.on_wait) if mmsi else []
                    if mm is not None and len(mm_waits) + len(waits) <= 1:
                        if waits or upds:
                            if mmsi is None:
                                mm.sync_info = mybir.SyncInfo(
                                    on_wait=waits, on_update=upds)
                            else:
                                mm.sync_info = mybir.SyncInfo(
                                    on_wait=mm_waits + waits,
                                    on_update=list(mmsi.on_update) + upds)
                        mm.merge_dependencies_from(inst)
                        remove.add(i)
                        n_removed += 1
                        i += 1
                        continue
                last_key = key
            elif tn == "InstMatmult":
                if getattr(inst, "is_transpose", False):
                    last_key = None
            i += 1
        if remove:
            removed_names = {insts[k].name for k in remove}
            b.instructions = [x for k, x in enumerate(insts) if k not in remove]
            for bb in nc.main_func.blocks:
                for x in bb.instructions:
                    for nm in removed_names:
                        try:
                            x.try_remove_dependency(nm)
                        except Exception:
                            pass
    return n_removed


def build_program(R, B_total):
    """Build the per-core Bass program. R = batch rows per core."""
    import contextlib

    import concourse.bacc as bacc
    import concourse.mybir as mybir
    import concourse.tile as tile

    f32 = mybir.dt.float32
    bf16 = mybir.dt.bfloat16
    Alu = mybir.AluOpType
    Act = mybir.ActivationFunctionType

    assert R % 512 == 0
    NT = R // 512
    KT = [D_IN // 128, D_H // 128, D_H // 128]
    MT = [D_H // 128, D_H // 128, D_OUT // 128]
    assert MT[2] == 8 and KT[0] == 8 and NT == 4, "schedule tuned for full size"
    inv_B = 1.0 / float(B_total)
    GROUP = [list(range(N_CORES))]
    HEAD = 2                      # strips interleaved at L1/L2 start
    SPLIT = [KT[l] - 2 for l in range(3)]  # deferred k-tail for head strips
    CHB = [[0, 8, 12, 14, MT[0]],
           [0, 8, 12, 14, MT[1]],
           [0, 3, 5, 7, 8]]   # L2 output-strip chunks

    nc = bacc.Bacc(None, num_devices=N_CORES)

    xt_d = nc.dram_tensor("xt", [D_IN, R], bf16, kind="ExternalInput")
    # weights pre-tiled strip-major on host: [m_strip, partition(k%128), k//128*128+f]
    w0_d = nc.dram_tensor("w0t", [MT[0], 128, KT[0] * 128], bf16, kind="ExternalInput")
    w1_d = nc.dram_tensor("w1t", [MT[1], 128, KT[1] * 128], bf16, kind="ExternalInput")
    w2_d = nc.dram_tensor("w2t", [MT[2], 128, KT[2] * 128], bf16, kind="ExternalInput")
    g0_d = nc.dram_tensor("g0", [D_H], f32, kind="ExternalInput")
    g1_d = nc.dram_tensor("g1", [D_H], f32, kind="ExternalInput")
    g2_d = nc.dram_tensor("g2", [D_OUT], f32, kind="ExternalInput")
    b2_d = nc.dram_tensor("beta2", [D_OUT], f32, kind="ExternalInput")
    out_d = nc.dram_tensor("out", [D_OUT, R], bf16, kind="ExternalOutput")

    cc_in = [[nc.dram_tensor(f"cc_in{l}_{q}", [128, 2 * (b - a)], f32)
              for q, (a, b) in enumerate(zip(CHB[l], CHB[l][1:]))]
             for l in range(3)]
    cc_out = [[nc.dram_tensor(f"cc_out{l}_{q}", [128, 2 * (b - a)], f32,
                              addr_space="Shared")
               for q, (a, b) in enumerate(zip(CHB[l], CHB[l][1:]))]
              for l in range(3)]

    with tile.TileContext(nc) as tc:
        with contextlib.ExitStack() as ctx:
            act = ctx.enter_context(tc.tile_pool(name="act", bufs=32))
            w0p = ctx.enter_context(tc.tile_pool(name="w0p", bufs=8))
            wp = ctx.enter_context(tc.tile_pool(name="wp", bufs=4))
            pspool = ctx.enter_context(tc.tile_pool(name="ps", bufs=8, space="PSUM"))
            small = ctx.enter_context(tc.tile_pool(name="small", bufs=1))

            # ---------- initial loads ----------
            xt_r = xt_d[:].rearrange("(j p) r -> p j r", p=128)
            H = R // 2
            w0s = []
            for m in range(2):
                t = w0p.tile([128, KT[0] * 128], bf16, tag="w0", name=f"w0_{m}")
                nc.gpsimd.dma_start(out=t, in_=w0_d[m])
                w0s.append(t)
            GP_X = {2, 5}  # these x k-tiles ride the gpsimd ring whole
            xts = []
            for j in range(KT[0]):
                t = act.tile([128, R], bf16, tag="act", name=f"xt{j}")
                if j in GP_X:
                    nc.gpsimd.dma_start(out=t, in_=xt_r[:, j, :])
                else:
                    nc.sync.dma_start(out=t[:, 0:H], in_=xt_r[:, j, 0:H])
                    nc.scalar.dma_start(out=t[:, H:R], in_=xt_r[:, j, H:R])
                xts.append(t)
            for m in range(2, MT[0]):
                t = w0p.tile([128, KT[0] * 128], bf16, tag="w0", name=f"w0_{m}")
                nc.gpsimd.dma_start(out=t, in_=w0_d[m])
                w0s.append(t)

            g_t = []
            for l, gd in enumerate((g0_d, g1_d, g2_d)):
                t = small.tile([128, MT[l]], f32, tag=f"g{l}", name=f"g{l}")
                nc.gpsimd.dma_start(out=t, in_=gd[:].rearrange("(m p) -> p m", p=128))
                g_t.append(t)
            b2_t = small.tile([128, MT[2]], f32, tag="b2", name="b2")
            nc.gpsimd.dma_start(out=b2_t, in_=b2_d[:].rearrange("(m p) -> p m", p=128))

            eps_t = small.tile([128, 1], f32, tag="eps", name="eps")
            nc.vector.memset(eps_t, BN_EPS)
            # prime the Act engine's sqrt table outside the critical path
            dummy = small.tile([128, 1], f32, tag="dummy", name="dummy")
            nc.vector.memset(dummy, 1.0)
            nc.scalar.activation(out=dummy, in_=dummy, func=Act.Sqrt,
                                 bias=eps_t[:, 0:1])

            # w1 strip prefetch (behind w0 on the gpsimd ring)
            w1s = {}

            def w1_tile(m):
                if m not in w1s:
                    t = wp.tile([128, KT[1] * 128], bf16, tag="w", name=f"w1_{m}")
                    nc.gpsimd.dma_start(out=t, in_=w1_d[m])
                    w1s[m] = t
                return w1s[m]

            w2s = {}

            def w2_tile(m):
                if m not in w2s:
                    t = wp.tile([128, KT[2] * 128], bf16, tag="w", name=f"w2_{m}")
                    nc.gpsimd.dma_start(out=t, in_=w2_d[m])
                    w2s[m] = t
                return w2s[m]

            for m in range(3):
                w1_tile(m)

            BN = [small.tile([128, MT[l] * NT * 6], f32, tag=f"BN{l}", name=f"BN{l}")
                  for l in range(3)]

            # ---------- stats helpers ----------
            def part1(l, q):
                """pre-AR: aggregate chunk stats, form S/Q sums, AllReduce."""
                m0, m1 = CHB[l][q], CHB[l][q + 1]
                mh = m1 - m0
                mv = small.tile([128, mh, 2], f32, tag=f"mv{l}{q}", name=f"mv{l}{q}")
                for m in range(m0, m1):
                    nc.vector.bn_aggr(out=mv[:, m - m0, :],
                                      in_=BN[l][:, m * NT * 6:(m + 1) * NT * 6])
                # S = mean*R ; Q = (var + mean^2)*R  (exact cross-core sums)
                sf = small.tile([128, 2, mh], f32, tag=f"sf{l}{q}", name=f"sf{l}{q}")
                nc.vector.tensor_scalar_mul(sf[:, 0, :], mv[:, :, 0], float(R))
                nc.vector.tensor_mul(sf[:, 1, :], mv[:, :, 0], mv[:, :, 0])
                nc.vector.tensor_add(sf[:, 1, :], sf[:, 1, :], mv[:, :, 1])
                nc.vector.tensor_scalar_mul(sf[:, 1, :], sf[:, 1, :], float(R))
                nc.sync.dma_start(out=cc_in[l][q][:], in_=sf)
                nc.gpsimd.collective_compute(
                    "AllReduce", Alu.add, replica_groups=GROUP,
                    ins=[cc_in[l][q][:]], outs=[cc_out[l][q][:]])

            def part2(l, q, want_c):
                """post-AR: readback -> mean/var -> a = gamma*rstd [, c]."""
                m0, m1 = CHB[l][q], CHB[l][q + 1]
                mh = m1 - m0
                sg = small.tile([128, 2, mh], f32, tag=f"sg{l}{q}", name=f"sg{l}{q}")
                nc.sync.dma_start(
                    out=sg, in_=cc_out[l][q][:].rearrange("p (s m) -> p s m", s=2))
                mean = small.tile([128, mh], f32, tag=f"mean{l}{q}", name=f"mean{l}{q}")
                var = small.tile([128, mh], f32, tag=f"var{l}{q}", name=f"var{l}{q}")
                tmp = small.tile([128, mh], f32, tag=f"tmp{l}{q}", name=f"tmp{l}{q}")
                nc.vector.tensor_scalar_mul(mean, sg[:, 0, :], inv_B)
                nc.vector.tensor_scalar_mul(var, sg[:, 1, :], inv_B)
                nc.vector.tensor_mul(tmp, mean, mean)
                nc.vector.tensor_sub(var, var, tmp)
                nc.scalar.activation(out=var, in_=var, func=Act.Sqrt,
                                     bias=eps_t[:, 0:1])
                nc.vector.reciprocal(out=var, in_=var)
                a = small.tile([128, mh], f32, tag=f"a{l}{q}", name=f"a{l}{q}")
                nc.vector.tensor_mul(a, var, g_t[l][:, m0:m1])
                if not want_c:
                    return a, None
                c = small.tile([128, mh], f32, tag=f"c{l}{q}", name=f"c{l}{q}")
                nc.vector.tensor_mul(tmp, a, mean)
                nc.vector.tensor_sub(c, b2_t[:, m0:m1], tmp)
                return a, c

            def post(l, q, strips):
                """part2 + in-place u *= a for the chunk; 3/4 DVE, 1/4 Act."""
                a, _ = part2(l, q, False)
                m0, m1 = CHB[l][q], CHB[l][q + 1]
                for m in range(m0, m1):
                    s = strips[m][:]
                    ac = a[:, m - m0:m - m0 + 1]
                    if m % 4 == 3:
                        nc.scalar.activation(out=s, in_=s, func=Act.Copy, scale=ac)
                    else:
                        nc.vector.tensor_scalar_mul(s, s, ac)

            # ---------- matmul helpers ----------
            def w_ap_of(t):
                return lambda j: t[:, j * 128:(j + 1) * 128]

            def strips_rhs(strips):
                return lambda j, n: strips[j][:, n * 512:(n + 1) * 512]

            def mm_group(l, pss, w_ap, rhs, j):
                for n in range(NT):
                    nc.tensor.matmul(pss[n], w_ap(j), rhs(j, n),
                                     start=(j == 0), stop=(j == KT[l] - 1),
                                     skip_group_check=True)

            def consumers(l, m, pss, dest_at):
                for n in range(NT):
                    idx = m * NT + n
                    nc.scalar.activation(out=dest_at(m, n), in_=pss[n],
                                         func=Act.Copy)
                    nc.vector.bn_stats(out=BN[l][:, idx * 6:idx * 6 + 6],
                                       in_=pss[n])

            def ps_tiles(l, m):
                return [pspool.tile([128, 512], f32, tag="ps",
                                    name=f"ps{l}_{m}_{n}") for n in range(NT)]

            # ================= layer 0 (plain m loop) =================
            u0 = [act.tile([128, R], bf16, tag="act", name=f"u0_{m}")
                  for m in range(MT[0])]

            def u0_at(m, n):
                return u0[m][:, n * 512:(n + 1) * 512]

            rhs0 = strips_rhs(xts)
            for m in range(MT[0]):
                pss = ps_tiles(0, m)
                wap = w_ap_of(w0s[m])
                for j in range(KT[0]):
                    mm_group(0, pss, wap, rhs0, j)
                consumers(0, m, pss, u0_at)
                if m == 7:
                    part1(0, 0)
                if m == 11:
                    part1(0, 1)
                if m == 12:
                    post(0, 0, u0)
                if m == 13:
                    part1(0, 2)
                if m == 14:
                    post(0, 1, u0)

            # ================= layer 1 (interleaved head) =================
            part1(0, 3)  # L0 final chunk: start its AllReduce ASAP
            post(0, 2, u0)
            u1 = [act.tile([128, R], bf16, tag="act", name=f"u1_{m}")
                  for m in range(MT[1])]

            def u1_at(m, n):
                return u1[m][:, n * 512:(n + 1) * 512]

            rhs1 = strips_rhs(u0)
            head_ps = []
            for m in range(HEAD):
                w1_tile(m + 3)
                t = w1_tile(m)
                pss = ps_tiles(1, m)
                head_ps.append((pss, t))
                for j in range(SPLIT[1]):
                    mm_group(1, pss, w_ap_of(t), rhs1, j)
            post(0, 3, u0)  # deferred scales for u0[14..15]
            for m in range(HEAD):
                pss, t = head_ps[m]
                for j in range(SPLIT[1], KT[1]):
                    mm_group(1, pss, w_ap_of(t), rhs1, j)
                consumers(1, m, pss, u1_at)
            for m in range(HEAD, MT[1]):
                if m + 3 < MT[1]:
                    w1_tile(m + 3)
                else:
                    w2_tile(m + 3 - MT[1])  # prefetch w2 strips 0..2
                pss = ps_tiles(1, m)
                wap = w_ap_of(w1_tile(m))
                for j in range(KT[1]):
                    mm_group(1, pss, wap, rhs1, j)
                consumers(1, m, pss, u1_at)
                if m == 7:
                    part1(1, 0)
                if m == 11:
                    part1(1, 1)
                if m == 12:
                    post(1, 0, u1)
                if m == 13:
                    part1(1, 2)
                if m == 14:
                    post(1, 1, u1)

            # ================= layer 2 (interleaved head, bf16 out) =========
            part1(1, 3)
            post(1, 2, u1)
            u2 = [act.tile([128, R], bf16, tag="act", name=f"u2_{m}")
                  for m in range(MT[2])]

            def u2_at(m, n):
                return u2[m][:, n * 512:(n + 1) * 512]

            def post2(q, last=False):
                """part2 + final affine (bf16, DVE) + writeback on two rings."""
                a, c = part2(2, q, True)
                m0, m1 = CHB[2][q], CHB[2][q + 1]
                for m in range(m0, m1):
                    am = a[:, m - m0:m - m0 + 1]
                    cm = c[:, m - m0:m - m0 + 1]
                    s = u2[m][:]
                    nc.vector.tensor_scalar(s, s, am, cm, Alu.mult, Alu.add)
                    nc.sync.dma_start(
                        out=out_d[m * 128:(m + 1) * 128, 0:H], in_=u2[m][:, 0:H])
                    nc.gpsimd.dma_start(
                        out=out_d[m * 128:(m + 1) * 128, H:R], in_=u2[m][:, H:R])

            rhs2 = strips_rhs(u1)
            head_ps = []
            for m in range(HEAD):
                w2_tile(m + 3)
                t = w2_tile(m)
                pss = ps_tiles(2, m)
                head_ps.append((pss, t))
                for j in range(SPLIT[2]):
                    mm_group(2, pss, w_ap_of(t), rhs2, j)
            post(1, 3, u1)  # deferred scales for u1[14..15]
            for m in range(HEAD):
                pss, t = head_ps[m]
                for j in range(SPLIT[2], KT[2]):
                    mm_group(2, pss, w_ap_of(t), rhs2, j)
                consumers(2, m, pss, u2_at)
            # boundary m -> (part1 chunk, post2 chunk) per CHB[2]=[0,3,5,7,8]
            bound = {2: (0, None), 4: (1, 0), 6: (2, 1), 7: (3, 2)}
            for m in range(HEAD, MT[2]):
                if m + 3 < MT[2]:
                    w2_tile(m + 3)
                pss = ps_tiles(2, m)
                wap = w_ap_of(w2_tile(m))
                if m == MT[2] - 1:
                    # last strip: k-chain per psum tile so stats/copies of
                    # early tiles start before the layer's final matmul
                    for n in range(NT):
                        for j in range(KT[2]):
                            nc.tensor.matmul(
                                pss[n], wap(j), rhs2(j, n),
                                start=(j == 0), stop=(j == KT[2] - 1),
                                skip_group_check=True)
                        idx = m * NT + n
                        nc.scalar.activation(out=u2_at(m, n), in_=pss[n],
                                             func=Act.Copy)
                        nc.vector.bn_stats(
                            out=BN[2][:, idx * 6:idx * 6 + 6], in_=pss[n])
                else:
                    for j in range(KT[2]):
                        mm_group(2, pss, wap, rhs2, j)
                    consumers(2, m, pss, u2_at)
                if m in bound:
                    p1q, p2q = bound[m]
                    part1(2, p1q)
                    if p2q is not None:
                        post2(p2q)
            post2(len(CHB[2]) - 2, last=True)

    dedup_ldweights(nc, mybir)
    nc.compile()
    return nc


def _get_program(R, B_total):
    key = (R, B_total)
    if key not in _PROG_CACHE:
        _PROG_CACHE[key] = build_program(R, B_total)
    return _PROG_CACHE[key]


def prep_inputs(x, W0, W1, W2, gamma0, gamma1, gamma2, beta2, n_cores=N_CORES):
    """Host-side: transpose, cast to bf16, pre-tile weights, shard batch."""
    bf = ml_dtypes.bfloat16

    def strip_tiles(W):
        # W [F, K] -> [F//128 strips, 128 partitions(k%128), (K//128)*128] bf16
        # element [m, p, j*128+f] = W[m*128+f, j*128+p]
        F, Kd = W.shape
        wt = W.T.reshape(Kd // 128, 128, F // 128, 128)  # [j, p, m, f]
        return np.ascontiguousarray(wt.transpose(2, 1, 0, 3)).reshape(
            F // 128, 128, Kd // 128 * 128
        ).astype(bf)

    xT = np.ascontiguousarray(x.T)  # [D_IN, B]
    R = x.shape[0] // n_cores
    w0t = strip_tiles(np.asarray(W0, dtype=np.float32))
    w1t = strip_tiles(np.asarray(W1, dtype=np.float32))
    w2t = strip_tiles(np.asarray(W2, dtype=np.float32))
    g0 = np.ascontiguousarray(gamma0, dtype=np.float32)
    g1 = np.ascontiguousarray(gamma1, dtype=np.float32)
    g2 = np.ascontiguousarray(gamma2, dtype=np.float32)
    b2 = np.ascontiguousarray(beta2, dtype=np.float32)
    in_maps = []
    for c in range(n_cores):
        in_maps.append(
            {
                "xt": np.ascontiguousarray(xT[:, c * R:(c + 1) * R]).astype(bf),
                "w0t": w0t,
                "w1t": w1t,
                "w2t": w2t,
                "g0": g0,
                "g1": g1,
                "g2": g2,
                "beta2": b2,
            }
        )
    return in_maps, R


def kernel(
    x,
    W0,
    b0,
    gamma0,
    beta0,
    W1,
    b1,
    gamma1,
    beta1,
    W2,
    b2,
    gamma2,
    beta2,
):
    """Full-input entry point: shard across 8 NeuronCores, run, gather.

    b0/b1/b2/beta0/beta1 cancel exactly under training-mode BatchNorm
    (shift invariance), so they are not shipped to the device.
    """
    global LAST_RESULTS
    from concourse.bass_utils import run_bass_kernel_spmd

    x = np.asarray(x, dtype=np.float32)
    B = x.shape[0]
    in_maps, R = prep_inputs(
        x, np.asarray(W0), np.asarray(W1), np.asarray(W2),
        np.asarray(gamma0), np.asarray(gamma1), np.asarray(gamma2),
        np.asarray(beta2),
    )
    nc = _get_program(R, B)
    res = run_bass_kernel_spmd(nc, in_maps, core_ids=list(range(N_CORES)))
    LAST_RESULTS = res
    out = np.empty((B, D_OUT), dtype=np.float32)
    for c in range(N_CORES):
        out[c * R:(c + 1) * R, :] = res.results[c]["out"].T.astype(np.float32)
    return out
